# revision 48
# baseline (speedup 1.0000x reference)
"""Trainium2 Bass kernel for quantized cosine-distance (1 - cos similarity).

Math: the reference L2-normalizes both matrices, 7-bit-quantizes them with a
global scale and recombines 9 bit-sliced GEMMs - exactly round(xn*127/s) @
round(wn*127/s)^T * s_x*s_w, i.e. cosine similarity with ~1e-3 quantization
noise.  The harness gate is rel_err < 2e-2 against that reference, so any
quantization of comparable fidelity passes.  This kernel:
  - quantizes the x side as a pure dtype cast to fp8 e4m3 (no prescaling):
    half the casts run on the Activation engine, half as SWDGE cast-DMAs
    (gpsimd-issued SBUF->SBUF DMAs may convert dtypes), so qx never waits
    for the norm computation;
  - quantizes the w side prenormalized, qw = fp8(w * 512/||w||), via
    DVE/Pool tensor multiplies against a Pool-broadcast 512/||w|| row;
  - computes the GEMM with fp8 DoubleRow matmuls (256-deep contraction per
    instruction, 4x bf16 throughput, 0.5 cycles/row);
  - epilogue is then a pure per-partition scale (psum partitions = x rows):
    int8 q = round(psum * -512/(512*||x_b||)) = round(-cos * 512), running
    on ACT (activation scale) and DVE (tensor_scalar) - GPSIMD cannot read
    PSUM, so Pool takes quant/broadcast work instead.
Host decodes out = 1 + q/512.  Error vs the reference is ~8e-3 max,
dominated by the fp8 mantissa width; the int8 wire format adds <1e-3.

Norms on device: squares (bf16, DVE 2x mode) -> ones-vector matmuls
accumulate column sums of squares in PSUM -> reciprocal (DVE) -> sqrt (ACT).
The w-side row is partition-broadcast on Pool; the x-side row is bounced
through DRAM with a strided DMA to land transposed as a [128, 16]
per-partition scalar table.

Sharding: 2x4 grid over 8 cores - x split in 2 row-halves, weight in 4
row-quarters; each core computes a [2048, 2048] block of the [4096, 8192]
output, minimizing per-core HBM traffic (4 MB x + 4 MB w bf16 in, 4 MB int8
out; the model serializes all DMA at ~360 GB/s).  Main matmuls sweep
m-chunk-outer in three phases so each phase only needs the input quarters
already loaded; junk "filler" matmuls at phase boundaries keep the PE
p-state ramp at full clock; all bulk DMAs are >=256 KB so the shared
descriptor generator is never the bottleneck.
"""

import numpy as np
import ml_dtypes

import concourse.bass as bass
import concourse.mybir as mybir
import concourse.tile as tile
from concourse import bacc
from concourse.bass_utils import run_bass_kernel_spmd

F32 = mybir.dt.float32
BF16 = mybir.dt.bfloat16
FP8 = mybir.dt.float8e4
I8 = mybir.dt.int8
AF = mybir.ActivationFunctionType
ALU = mybir.AluOpType
PM = mybir.MatmulPerfMode
P = 128

B_FULL, D_FULL, M_FULL = 4096, 1024, 8192
GB, GM = 2, 4                      # core grid: 2 b-groups x 4 m-groups
BC = B_FULL // GB                  # 2048 b-columns per core
MC = M_FULL // GM                  # 2048 m-columns per core
KB = D_FULL // P                   # 8 contraction subtiles of 128
KO = 512.0                         # int8 output scale: q = round(-cos*KO)
KQW = 512.0                        # w-side prenorm quant scale (power of 2)
N_CORES = GB * GM
NBB = BC // P                      # 16 b-blocks per core

LAST = {}
_PROGRAM_CACHE = {}


def _run_spmd(nc, in_maps, core_ids, **kw):
    """run_bass_kernel_spmd with one retry - the axon-tunneled devices
    occasionally report NRT_EXEC_UNIT_UNRECOVERABLE transiently."""
    import time as _time

    try:
        return run_bass_kernel_spmd(nc, in_maps, core_ids=core_ids, **kw)
    except Exception:
        _time.sleep(90.0)
        return run_bass_kernel_spmd(nc, in_maps, core_ids=core_ids, **kw)


def build_program(
    n_warm=8,
    # engine assignment patterns per op class, cycled in emission order:
    # d=DVE, a=ACT, p=Pool
    cast_engines="ma",
    sq_engines="d",
    epi_engines="aad",
    quant_engines="dp",
    phase_order=0,
    n_fill=8,
    load_order=0,
    n_fill2=0,
    split_stores=True,
    mm_bufs=6,
    ssq_bufs=2,
    interleave_p23=False,
):
    nc = bacc.Bacc("TRN2", target_bir_lowering=False, debug=False)
    xT = nc.dram_tensor("xT", [D_FULL, BC], BF16, kind="ExternalInput")
    wT = nc.dram_tensor("wT", [D_FULL, MC], BF16, kind="ExternalInput")
    qout = nc.dram_tensor("qout", [BC, MC], I8, kind="ExternalOutput")

    def eng(c):
        return {"d": nc.vector, "p": nc.gpsimd}[c]

    with tile.TileContext(nc) as tc:
        with (
            tc.tile_pool(name="const", bufs=1) as cpool,
            tc.tile_pool(name="ld", bufs=17) as ldp,
            tc.tile_pool(name="sq", bufs=5) as sqp,
            tc.tile_pool(name="q", bufs=1) as qp,
            tc.tile_pool(name="misc", bufs=1) as misc,
            tc.tile_pool(name="outp", bufs=17) as outp,
            tc.tile_pool(name="dram", bufs=1, space="DRAM") as dram,
            tc.tile_pool(name="psum", bufs=1, space="PSUM") as psp,
        ):
            # PE warmup: junk matmuls so the p-state ramp completes during
            # the load phase (model: full clock after 3us continuous busy)
            warm = cpool.tile([P, 512], BF16)
            nc.vector.memset(warm[:], 1.0)
            wps = psp.tile([P, 512], F32, tag="mm", bufs=mm_bufs, name="warmps")
            for i in range(n_warm):
                nc.tensor.matmul(
                    wps[:], warm[:, 0:P], warm[:], start=True, stop=True
                )

            ones = cpool.tile([P, 1], BF16)
            nc.vector.memset(ones[:], 1.0)

            # ---- loads: [128, 1024] bf16 tiles; quarter order = phase order
            QUARTERS = (
                (("w", 0), ("x", 0), ("w", 1), ("x", 1)),
                (("w", 0), ("x", 0), ("x", 1), ("w", 1)),
            )[load_order % 2]
            ld = {}
            srcs = {"w": wT, "x": xT}
            if load_order == 2:
                # h1 halves interleaved g-wise: w and x second halves arrive
                # together instead of strictly w-then-x
                order = (
                    [("w", 0, g) for g in range(KB // 2)]
                    + [("x", 0, g) for g in range(KB // 2)]
                    + [it for g in range(KB // 2)
                       for it in (("w", 1, g), ("x", 1, g))]
                )
            else:
                order = [(s_, h_, g) for s_, h_ in QUARTERS
                         for g in range(KB // 2)]
            for side, h, g in order:
                if True:
                    t = ldp.tile([P, 2, 1024], BF16, tag="ld",
                                 name=f"ld{side}{g}_{h}")
                    src = srcs[side][
                        2 * g * P : (2 * g + 2) * P,
                        h * 1024 : (h + 1) * 1024,
                    ]
                    nc.sync.dma_start(
                        t[:], src.rearrange("(j p) c -> p j c", p=P)
                    )
                    ld[(side, g, h)] = t

            qx = qp.tile([P, KB, BC], FP8, tag="qx")
            qw = qp.tile([P, KB, MC], FP8, tag="qw")
            qt = {"x": qx, "w": qw}

            cbw = misc.tile([P, MC], BF16, tag="cbw", name="cbw")
            rnw_bf = misc.tile([1, MC], BF16, tag="rnwb", name="rnwb")
            rec = {
                "x": misc.tile([1, BC], F32, tag="recx", name="recx"),
                "w": misc.tile([1, MC], F32, tag="recw", name="recw"),
            }
            rnx_f = misc.tile([1, BC], F32, tag="rnxf", name="rnxf")
            rnx_dram = dram.tile([NBB, P], F32, name="rnxd")
            rnxp = misc.tile([P, NBB], F32, tag="rnxp", name="rnxp")
            rnxs = misc.tile([P, NBB], F32, tag="rnxs", name="rnxs")

            cast_i = [0]
            sq_i = [0]
            quant_i = [0]

            def casts(side, h):
                hsl = slice(h * 1024, (h + 1) * 1024)
                for g in range(KB // 2):
                    ce = cast_engines[cast_i[0] % len(cast_engines)]
                    dst = qt[side][:, 2 * g : 2 * g + 2, hsl]
                    if ce == "a":
                        nc.scalar.activation(
                            dst, ld[(side, g, h)][:], AF.Copy
                        )
                    elif ce == "m":
                        # SWDGE cast-DMA: bf16 SBUF -> fp8 SBUF via the DMA
                        # engines (only gpsimd-issued DMAs may cast)
                        nc.gpsimd.dma_start(dst, ld[(side, g, h)][:])
                    elif ce == "M":
                        # cast-load straight from DRAM (bf16 -> fp8), so qx
                        # does not wait for the bf16 staging tiles
                        src = srcs[side][
                            2 * g * P : (2 * g + 2) * P,
                            h * 1024 : (h + 1) * 1024,
                        ]
                        nc.gpsimd.dma_start(
                            dst, src.rearrange("(j p) c -> p j c", p=P)
                        )
                    else:
                        eng(ce).tensor_scalar_mul(dst, ld[(side, g, h)][:], 1.0)
                    cast_i[0] += 1

            def norms(side, h):
                sqs = []
                for g in range(KB // 2):
                    s = sqp.tile([P, 2, 1024], BF16, tag="sq",
                                 name=f"sq{side}{h}_{g}")
                    src = ld[(side, g, h)][:]
                    se = sq_engines[sq_i[0] % len(sq_engines)]
                    if se == "a":
                        nc.scalar.square(s[:], src)
                    else:
                        eng(se).tensor_mul(s[:], src, src)
                    sq_i[0] += 1
                    sqs.append(s)
                for sub in range(2):
                    ch = 2 * h + sub
                    sl = slice(ch * 512, ch * 512 + 512)
                    ssl = slice(sub * 512, sub * 512 + 512)
                    ssq = psp.tile([1, 512], F32, tag="ssq", bufs=ssq_bufs,
                                   name=f"ssq{side}{ch}")
                    for k in range(KB):
                        nc.tensor.matmul(
                            ssq[:], ones[:], sqs[k // 2][:, k % 2, ssl],
                            start=(k == 0), stop=(k == KB - 1),
                        )
                    # the scalar chain gates all epilogues of its quarter -
                    # mark highest priority so the scheduler slots the tiny
                    # ops as soon as their deps resolve
                    with tc.high_priority():
                        nc.vector.reciprocal(rec[side][:, sl], ssq[:])
                        if side == "w":
                            # 512*rsqrt: w rows are quantized prenormalized,
                            # qw = fp8(w * 512/||w||), so the epilogue is a
                            # pure per-partition scale (ACT-compatible)
                            nc.scalar.activation(
                                rnw_bf[:, sl], rec[side][:, sl], AF.Sqrt,
                                scale=KQW * KQW,
                            )
                            nc.gpsimd.partition_broadcast(
                                cbw[:, sl], rnw_bf[0:1, sl]
                            )
                        else:
                            nc.scalar.activation(
                                rnx_f[:, sl], rec[side][:, sl], AF.Sqrt
                            )
                if side == "w":
                    hsl = slice(h * 1024, (h + 1) * 1024)
                    for k in range(KB):
                        qe = quant_engines[quant_i[0] % len(quant_engines)]
                        eng(qe).tensor_mul(
                            qw[:, k, hsl],
                            ld[(side, k // 2, h)][:, k % 2, :],
                            cbw[:, hsl],
                        )
                        quant_i[0] += 1
                if side == "x":
                    # transpose 1/||x_row|| into per-partition layout:
                    # [1, 1024] -> DRAM [8, 128] -> strided load [128, 8]
                    hsl = slice(h * 1024, (h + 1) * 1024)
                    dsl = slice(h * 8, (h + 1) * 8)
                    with tc.high_priority():
                        nc.sync.dma_start(rnx_dram[dsl, :], rnx_f[:, hsl])
                        nc.sync.dma_start(
                            rnxp[:, dsl], rnx_dram[dsl, :].transpose([1, 0])
                        )
                        nc.vector.tensor_scalar_mul(
                            rnxs[:, dsl], rnxp[:, dsl], -KO / KQW
                        )

            # ---- main GEMM sweeps.  Phases (emission order = execution
            # order per engine queue):
            #   P1: wpair 0 x bb 0..7   (needs w-h0 + x-h0 casts)
            #   P2: wpair 1 x bb 0..7   (+ w-h1)  -> bb 0..7 stored
            #   P3: wpair 0,1 x bb 8..15 (+ x-h1)
            ots = [
                outp.tile([P, MC], I8, tag="ot", name=f"ot{bb}")
                for bb in range(NBB)
            ]
            epi_i = [0]
            done_w = [0] * NBB

            def mains(wpairs, bbs):
                for bb in bbs:
                    for wpair, half in [
                        (wp, hf) for wp in wpairs for hf in range(2)
                    ]:
                        mcol = wpair * 1024 + half * 512
                        ps = psp.tile([P, 512], F32, tag="mm", bufs=mm_bufs,
                                      name=f"mm{bb}_{wpair}_{half}")
                        for g in range(KB // 2):
                            nc.tensor.matmul(
                                ps[:],
                                qx[:, 2 * g : 2 * g + 2, bb * P : (bb + 1) * P],
                                qw[:, 2 * g : 2 * g + 2, mcol : mcol + 512],
                                start=(g == 0), stop=(g == KB // 2 - 1),
                                perf_mode=PM.DoubleRow,
                            )
                        e = epi_engines[epi_i[0] % len(epi_engines)]
                        osl = ots[bb][:, mcol : mcol + 512]
                        if e == "a":
                            nc.scalar.activation(
                                osl, ps[:], AF.Copy,
                                scale=rnxs[:, bb : bb + 1],
                            )
                        else:
                            nc.vector.tensor_scalar_mul(
                                osl, ps[:], rnxs[:, bb : bb + 1]
                            )
                        epi_i[0] += 1
                        done_w[bb] += 1
                        if split_stores and done_w[bb] in (2, 4):
                            hp = (done_w[bb] - 2) // 2
                            if wpair == hp:
                                nc.sync.dma_start(
                                    qout[bb * P : (bb + 1) * P,
                                         hp * 1024 : (hp + 1) * 1024],
                                    ots[bb][:, hp * 1024 : (hp + 1) * 1024],
                                )
                    if not split_stores and done_w[bb] == 4:
                        nc.sync.dma_start(
                            qout[bb * P : (bb + 1) * P, :], ots[bb][:]
                        )

            def filler(n):
                # junk matmuls that are always ready: absorb what would be
                # PE idle (which resets the p-state ramp to half clock)
                for _ in range(n):
                    nc.tensor.matmul(
                        wps[:], warm[:, 0:P], warm[:], start=True, stop=True
                    )

            if phase_order == 0:
                norms("w", 0)
                filler(n_fill2)
                casts("x", 0)
                norms("x", 0)
                filler(n_fill2)
                mains((0,), range(8))
                norms("w", 1)
                if interleave_p23:
                    casts("x", 1)
                    norms("x", 1)
                    filler(n_fill)
                    for bb in range(8):
                        mains((1,), [bb])
                        mains((0,), [bb + 8])
                    mains((1,), range(8, 16))
                else:
                    filler(n_fill)
                    mains((1,), range(8))
                    casts("x", 1)
                    norms("x", 1)
                    filler(n_fill)
                    mains((0, 1), range(8, 16))
            else:
                # loads/pipes ordered w0, x0, x1, w1: sweep wpair0 over all
                # b-blocks first, wpair1 (gated by the last-loaded w half)
                # last
                norms("w", 0)
                casts("x", 0)
                norms("x", 0)
                mains((0,), range(8))
                casts("x", 1)
                norms("x", 1)
                filler(n_fill)
                mains((0,), range(8, 16))
                norms("w", 1)
                filler(n_fill)
                mains((1,), range(16))
    nc.compile()
    return nc


def _f32(a):
    return np.ascontiguousarray(np.asarray(a, dtype=np.float32))


def kernel(x, weight):
    x = _f32(x)
    w = _f32(weight)
    assert x.shape == (B_FULL, D_FULL) and w.shape == (M_FULL, D_FULL)

    nc = _PROGRAM_CACHE.get("main")
    if nc is None:
        nc = _PROGRAM_CACHE["main"] = build_program()

    bf = ml_dtypes.bfloat16
    xT = [
        np.ascontiguousarray(x[i * BC : (i + 1) * BC].T.astype(bf))
        for i in range(GB)
    ]
    wT = [
        np.ascontiguousarray(w[j * MC : (j + 1) * MC].T.astype(bf))
        for j in range(GM)
    ]
    in_maps = [
        {"xT": xT[c // GM], "wT": wT[c % GM]} for c in range(N_CORES)
    ]
    r = _run_spmd(nc, in_maps, core_ids=list(range(N_CORES)))
    LAST["nc"] = nc
    LAST["res"] = r

    out = np.empty((B_FULL, M_FULL), dtype=np.float32)
    for c in range(N_CORES):
        i, j = c // GM, c % GM
        q = r.results[c]["qout"].astype(np.float32)
        out[i * BC : (i + 1) * BC, j * MC : (j + 1) * MC] = 1.0 + q / KO
    return out
